# revision 33
# baseline (speedup 1.0000x reference)
"""Trainium2 Bass kernel for Memorynet (KNN-interp + 1x1-conv MLP).

Pure data parallel over batch (32 batches -> 8 cores x 4).

Per 128-token tile (threshold-mask design, no find_index8/scatter):
  E = S - |p1|^2 - eps = -(d2+eps)   one 24-row bf16-split matmul -> PSUM
  max8(E) -> top-8 neighbor scores v (DVE)
  group math: U = sum 1/v_k (k<3), lam = midpoint weight threshold (DVE)
  W = Reciprocal(E * U) on ACT (raw InstActivation; rel err ~1e-5)
      = normalized inverse-distance weights for ALL 512 candidates
  A = (W >= lam) * W  one scalar_tensor_tensor pass (DVE, 2x_2p mode)
  A.T via DMA transpose; recv@W1r fused as A @ G1 (G1 = f2@W1r.T, host)
MLP feature-major bf16 with BN+ReLU folded into ACT activations.
PE kept continuously busy (2-group software pipeline lag) to stay at
2.4 GHz p-state. Output bf16, host casts to fp32.
"""

import sys

sys.path.insert(0, "/opt/trn_rl_repo")

import numpy as np
import ml_dtypes

import concourse.bass as bass
import concourse.bacc as bacc_mod
import concourse.mybir as mybir
from concourse.tile import TileContext
from concourse.bass_utils import run_bass_kernel_spmd

EPS_DIST = 1e-6  # ranking/weight epsilon (safety net; min-cap handles flips)
EPS_BN = 1e-5
NCORES = 8
BPC = 4  # batches per core
N1, N2, C1, C2 = 2048, 512, 128, 256
CIN, H1, H2 = C1 + C2, 256, 128
NT = N1 // 128   # 16 token tiles / batch
NG_B = 4         # groups per batch (4 tiles each)
NGRP = BPC * NG_B  # 16 groups per core
KROWS = 24       # matmul contraction rows

f32 = mybir.dt.float32
bf16 = mybir.dt.bfloat16

AT = mybir.ActivationFunctionType
OP = mybir.AluOpType


def build_bass():
    nc = bacc_mod.Bacc()
    p1e = nc.declare_dram_parameter("p1e", [BPC, KROWS, N1], bf16, isOutput=False)
    rhs4 = nc.declare_dram_parameter("rhs4", [BPC, KROWS, N2], bf16, isOutput=False)
    f1T = nc.declare_dram_parameter("f1T", [BPC, C1, N1], bf16, isOutput=False)
    g1s = [
        nc.declare_dram_parameter(f"g1_{b}", [N2, H1], bf16, isOutput=False)
        for b in range(BPC)
    ]
    W1fd = nc.declare_dram_parameter("W1fT", [C1, H1], bf16, isOutput=False)
    W2Td = nc.declare_dram_parameter("W2T", [H1, H2], bf16, isOutput=False)
    sb1d = nc.declare_dram_parameter("sb1", [H1, 2], f32, isOutput=False)
    sb2d = nc.declare_dram_parameter("sb2", [H2, 2], f32, isOutput=False)
    outT = nc.declare_dram_parameter("outT", [BPC, H2, N1], bf16, isOutput=True)

    def act_recip(out_ap, in_ap, scale):
        """Raw Reciprocal activation: out = 1/(in * scale)."""
        eng = nc.scalar
        if isinstance(scale, float):
            sc = mybir.ImmediateValue(dtype=f32, value=scale)
        else:
            sc = eng.lower_ap(scale)
        inst = mybir.InstActivation(
            name=nc.get_next_instruction_name(),
            func=AT.Reciprocal,
            ins=[
                eng.lower_ap(in_ap),
                mybir.ImmediateValue(dtype=f32, value=0.0),  # bias
                sc,                                          # scale
                mybir.ImmediateValue(dtype=f32, value=0.0),  # alpha
            ],
            outs=[eng.lower_ap(out_ap)],
        )
        return eng.add_instruction(inst)

    with TileContext(nc) as tc:
        with (
            tc.tile_pool(name="const", bufs=1) as cpool,
            tc.tile_pool(name="batch", bufs=3) as bpool,
            tc.tile_pool(name="vg", bufs=3) as vpool,
            tc.tile_pool(name="wp", bufs=6) as wpool,
            tc.tile_pool(name="ap", bufs=3) as apool,
            tc.tile_pool(name="tp", bufs=3) as tpool,
            tc.tile_pool(name="hp", bufs=2) as hpool,
            tc.tile_pool(name="ps_s", bufs=6, space="PSUM") as ps_s,
            tc.tile_pool(name="ps_mlp", bufs=2, space="PSUM") as ps_mlp,
        ):
            # ---- constants ----
            W1f = cpool.tile([C1, H1], bf16)
            nc.sync.dma_start(out=W1f[:], in_=W1fd[:, :])
            W2T = [cpool.tile([128, H2], bf16, tag=f"w2_{k}", name=f"w2_{k}")
                   for k in range(2)]
            for k in range(2):
                nc.sync.dma_start(out=W2T[k][:], in_=W2Td[128 * k:128 * (k + 1), :])
            sb1 = [cpool.tile([128, 2], f32, tag=f"sb1_{k}", name=f"sb1_{k}")
                   for k in range(2)]
            for k in range(2):
                nc.sync.dma_start(out=sb1[k][:], in_=sb1d[128 * k:128 * (k + 1), :])
            sb2 = cpool.tile([128, 2], f32)
            nc.sync.dma_start(out=sb2[:], in_=sb2d[:, :])

            bstate = {}

            def load_batch(b):
                p1eb = bpool.tile([KROWS, N1], bf16, tag="p1eb")
                nc.sync.dma_start(out=p1eb[:], in_=p1e[b, :, :])
                rhsb = bpool.tile([KROWS, N2], bf16, tag="rhsb")
                nc.sync.dma_start(out=rhsb[:], in_=rhs4[b, :, :])
                g1sb = bpool.tile([128, 4, H1], bf16, tag="g1sb")
                nc.sync.dma_start(
                    out=g1sb[:], in_=g1s[b][:, :].rearrange("(c p) d -> p c d", p=128)
                )
                f1b = bpool.tile([C1, N1], bf16, tag="f1b")
                nc.sync.dma_start(out=f1b[:], in_=f1T[b, :, :])
                bstate[b] = (p1eb, rhsb, g1sb, f1b)

            gstate = {}   # g -> (v, Es)
            astate = {}   # g -> ATt

            def emit_pair(q):
                """S matmul + max8 for tiles 2q, 2q+1."""
                b = q // 8
                p1eb, rhsb = bstate[b][0], bstate[b][1]
                g = q // 2
                if q % 2 == 0:
                    v = vpool.tile([128, 4, 8], f32, tag="v")
                    gstate[g] = {"v": v, "W": [None] * 4}
                st = gstate[g]
                for i in range(2):
                    tau = 2 * q + i          # global tile 0..63
                    tt = tau % NT            # tile within batch
                    t = tt % 4               # tile within group
                    E = ps_s.tile([128, N2], f32, tag="E")
                    nc.tensor.matmul(
                        out=E[:],
                        lhsT=p1eb[:, 128 * tt:128 * (tt + 1)],
                        rhs=rhsb[:],
                        start=True,
                        stop=True,
                    )
                    nc.vector.max(out=st["v"][:, t, :], in_=E[:])
                    # unnormalized weights W = 1/(d2+eps) = recip(-E);
                    # no gmath dependency -> frees the PSUM bank fast
                    W = wpool.tile([128, N2], f32, tag="W")
                    act_recip(W[:], E[:], -1.0)
                    st["W"][t] = W

            def emit_group_mid(g):
                """gmath + ACT W + masked-keep + transpose for group g."""
                st = gstate[g]
                v = st["v"]
                # clamp v away from 0 so recip/U stay finite (flip safety)
                vv = vpool.tile([128, 4, 4], f32, tag="vv")
                nc.vector.tensor_scalar_min(vv[:], v[:, :, 0:4], -1e-7)
                r = vpool.tile([128, 4, 4], f32, tag="r")
                nc.vector.reciprocal(out=r[:], in_=vv[:])
                U = vpool.tile([128, 4], f32, tag="U")
                nc.vector.reduce_sum(
                    out=U[:], in_=r[:, :, 0:3], axis=mybir.AxisListType.X
                )
                negU = vpool.tile([128, 4], f32, tag="negU")
                nc.vector.tensor_scalar_mul(negU[:], U[:], -1.0)
                NUinv = vpool.tile([128, 4], f32, tag="NUinv")
                nc.vector.reciprocal(out=NUinv[:], in_=negU[:])
                lnum = vpool.tile([128, 4], f32, tag="lnum")
                nc.vector.tensor_tensor(
                    out=lnum[:], in0=r[:, :, 2], in1=r[:, :, 3], op=OP.add
                )
                lam = vpool.tile([128, 4], f32, tag="lam")
                nc.vector.tensor_scalar_mul(lam[:], lnum[:], -0.5)
                # A = min(NUinv * ((W>=lam)*W), 1): mask, normalize, cap
                Ag = apool.tile([128, 4, N2], bf16, tag="Ag")
                for t in range(4):
                    W = st["W"][t]
                    nc.vector.scalar_tensor_tensor(
                        out=Ag[:, t, :], in0=W[:], scalar=lam[:, t:t + 1],
                        in1=W[:], op0=OP.is_ge, op1=OP.mult,
                    )
                    nc.vector.tensor_scalar(
                        out=Ag[:, t, :], in0=Ag[:, t, :],
                        scalar1=NUinv[:, t:t + 1], scalar2=1.0,
                        op0=OP.mult, op1=OP.min,
                    )
                ATt = tpool.tile([128, 16, 128], bf16, tag="ATt")
                nc.sync.dma_start_transpose(out=ATt[:], in_=Ag[:])
                astate[g] = ATt
                gstate.pop(g)

            def emit_mlp(g):
                b, gb = g // NG_B, g % NG_B
                _, _, g1sb, f1b = bstate[b]
                ATt = astate.pop(g)
                ATv = ATt[:].rearrange("p (t c) r -> p c t r", c=4)
                f1g = f1b[:, 512 * gb:512 * (gb + 1)]
                h1 = [hpool.tile([128, 512], bf16, tag=f"h1_{m}", name=f"h1_{m}")
                      for m in range(2)]
                for m in range(2):
                    l1p = ps_mlp.tile([128, 512], f32, tag="mlp")
                    for c in range(4):
                        nc.tensor.matmul(
                            out=l1p[:],
                            lhsT=g1sb[:, c, 128 * m:128 * (m + 1)],
                            rhs=ATv[:, c],
                            start=(c == 0),
                            stop=False,
                        )
                    nc.tensor.matmul(
                        out=l1p[:],
                        lhsT=W1f[:, 128 * m:128 * (m + 1)],
                        rhs=f1g,
                        start=False,
                        stop=True,
                    )
                    nc.scalar.activation(
                        out=h1[m][:], in_=l1p[:], func=AT.Relu,
                        scale=sb1[m][:, 0:1], bias=sb1[m][:, 1:2],
                    )
                l2p = ps_mlp.tile([128, 512], f32, tag="mlp")
                for k in range(2):
                    nc.tensor.matmul(
                        out=l2p[:], lhsT=W2T[k][:], rhs=h1[k][:],
                        start=(k == 0), stop=(k == 1),
                    )
                o = hpool.tile([128, 512], bf16, tag="osb")
                nc.scalar.activation(
                    out=o[:], in_=l2p[:], func=AT.Relu,
                    scale=sb2[:, 0:1], bias=sb2[:, 1:2],
                )
                nc.sync.dma_start(out=outT[b, :, 512 * gb:512 * (gb + 1)], in_=o[:])

            # ---- software-pipelined emission ----
            load_batch(0)
            NSLOT = 32 + 7  # 32 S-pairs + drain for lagged MLP
            for s in range(NSLOT):
                # MLP first: its activations gate the PE's L2 matmuls, so
                # they must not queue behind this slot's W-recips on ACT
                if s >= 7 and (s - 7) % 2 == 0:
                    emit_mlp((s - 7) // 2)
                if s < 32:
                    if s % 8 == 6 and s // 8 + 1 < BPC:
                        load_batch(s // 8 + 1)
                    emit_pair(s)
                    if s % 2 == 1:
                        emit_group_mid(s // 2)
    nc.compile()
    return nc


_CACHE = {}


def _get_nc():
    if "nc" not in _CACHE:
        _CACHE["nc"] = build_bass()
    return _CACHE["nc"]


def _split3(x):
    a = x.astype(ml_dtypes.bfloat16)
    r = x - a.astype(np.float32)
    b = r.astype(ml_dtypes.bfloat16)
    c = (r - b.astype(np.float32)).astype(ml_dtypes.bfloat16)
    return a, b, c


def _prep_core(inputs, c):
    """Host-side prep of one core's input map (batches 4c..4c+4)."""
    sl = slice(BPC * c, BPC * (c + 1))
    p1 = inputs["points_1"][sl]     # [4, N1, 3]
    p2 = inputs["points_2"][sl]     # [4, N2, 3]
    f1 = inputs["features_1"][sl]   # [4, N1, C1]
    f2 = inputs["features_2"][sl]   # [4, N2, C2]

    p1T = np.transpose(p1, (0, 2, 1)).astype(np.float32)            # [4, 3, N1]
    p2T2 = (2.0 * np.transpose(p2, (0, 2, 1))).astype(np.float32)   # [4, 3, N2]
    p2sq = np.sum(p2.astype(np.float64) ** 2, -1)                   # [4, N2]
    a1, b1_, c1_ = _split3(p1T)
    x2, y2, z2 = _split3(p2T2)
    s1_, s2_, s3_ = _split3((-p2sq).astype(np.float32))
    p1sq = np.sum(p1.astype(np.float64) ** 2, -1) + EPS_DIST        # [4, N1]
    q1, q2, q3 = _split3((-p1sq).astype(np.float32)[:, None, :])    # [4, 1, N1]
    onesr = np.ones((BPC, 1, N1), ml_dtypes.bfloat16)
    onesc = np.ones((BPC, 1, N2), ml_dtypes.bfloat16)
    # E = 2*p1.p2 - |p2|^2 - |p1|^2 - eps  (= -(d2+eps))
    p1e = np.concatenate(
        [a1, a1, b1_, a1, b1_, c1_, onesr, onesr, onesr, q1, q2, q3], axis=1
    )  # [4, 24, N1]
    rhs4 = np.concatenate(
        [x2, y2, x2, z2, y2, x2,
         s1_[:, None, :], s2_[:, None, :], s3_[:, None, :],
         onesc, onesc, onesc], axis=1
    )  # [4, 24, N2]
    m = {
        "p1e": np.ascontiguousarray(p1e.astype(ml_dtypes.bfloat16)),
        "rhs4": np.ascontiguousarray(rhs4.astype(ml_dtypes.bfloat16)),
        "f1T": np.ascontiguousarray(
            np.transpose(f1, (0, 2, 1)).astype(ml_dtypes.bfloat16)
        ),
    }
    W1r = inputs["W1"][:, 0:C2]    # [H1, C2]
    W1fT = inputs["W1"][:, C2:].T  # [C1, H1]
    for b in range(BPC):
        g1b = f2[b].astype(np.float32) @ W1r.T.astype(np.float32)  # [N2, H1]
        m[f"g1_{b}"] = np.ascontiguousarray(g1b.astype(ml_dtypes.bfloat16))
    m["W1fT"] = np.ascontiguousarray(W1fT.astype(ml_dtypes.bfloat16))
    s1 = inputs["g1"] / np.sqrt(inputs["v1"] + EPS_BN)
    b1f = (inputs["b1"] - inputs["m1"]) * s1 + inputs["be1"]
    s2 = inputs["g2"] / np.sqrt(inputs["v2"] + EPS_BN)
    b2f = (inputs["b2"] - inputs["m2"]) * s2 + inputs["be2"]
    m["W2T"] = np.ascontiguousarray(inputs["W2"].T.astype(ml_dtypes.bfloat16))
    m["sb1"] = np.ascontiguousarray(np.stack([s1, b1f], -1).astype(np.float32))
    m["sb2"] = np.ascontiguousarray(np.stack([s2, b2f], -1).astype(np.float32))
    return m


def run(inputs, trace=False):
    nc = _get_nc()
    in_maps = [_prep_core(inputs, c) for c in range(NCORES)]
    res = run_bass_kernel_spmd(
        nc, in_maps, core_ids=list(range(NCORES)), trace=trace
    )
    outs = [np.asarray(r["outT"]).astype(np.float32) for r in res.results]
    full = np.concatenate(outs, 0)          # [32, H2, N1]
    out = np.ascontiguousarray(np.transpose(full, (0, 2, 1)))  # [32, N1, H2]
    return out, res


def kernel(**inputs):
    out, _ = run(inputs, trace=False)
    return out
